# revision 10
# baseline (speedup 1.0000x reference)
"""AFT simple attention (causal branch) on 8 TRN2 NeuronCores.

out = sigmoid(Q) * cumsum_L( (exp(K+mask) / cumsum_L(exp(K+mask))) * V )

Sharding: data-parallel over (batch n, head-block). Core c handles
n = c // 2 and heads [8*(c%2), 8*(c%2)+8).  Per-core shard = contiguous
[L=8192, 8*64=512] f32 slab per tensor; no cross-core communication.

On-chip layout: L on partitions (128 per tile), (h, e) on the free dim
(512).  All DMAs are fully contiguous.  The cumsum along L is computed
on the TensorEngine as triangular matmuls:
  - per tile: in-block inclusive scan  S_loc = U^T @ X   (U upper-tri ones)
  - per 8-tile chunk: column sums via indicator matmuls, an exclusive
    prefix over the 8 tile-sums via a small strictly-upper matmul, plus
    a running base B; the per-tile carry is broadcast into the same PSUM
    accumulation via selector matmuls.
Max-subtraction is skipped: Kw = exp(K-M)/cumsum(exp(K-M)) is exactly
invariant to M, and |K| <= ~6 so exp stays well within f32 range.
"""

from contextlib import ExitStack

import numpy as np

import concourse.bass as bass
import concourse.tile as tile
from concourse import bacc, mybir
from concourse.bass_utils import run_bass_kernel_spmd

N, L, H, E = 4, 8192, 16, 64
NCORES = 8
HPC = H // 2            # heads per core
FREE = HPC * E          # 512
P = 128                 # L positions per tile
NTILES = L // P         # 64
CHUNK = 8               # tiles per carry chunk
NCHUNKS = NTILES // CHUNK

F32 = mybir.dt.float32
F32R = mybir.dt.float32r
AF = mybir.ActivationFunctionType

_CACHE = {}


def _constants():
    U = np.triu(np.ones((P, P), dtype=np.float32))                 # U[j,l]=1 if j<=l
    IND = np.broadcast_to(
        np.eye(CHUNK, dtype=np.float32).reshape(-1), (P, CHUNK * CHUNK)
    ).copy()                                                       # IND[:,8j+m]=(m==j)
    SU = np.triu(np.ones((CHUNK, CHUNK), dtype=np.float32), k=1)   # strict upper
    ONES8 = np.ones((CHUNK, CHUNK), dtype=np.float32)
    SEL = np.kron(np.eye(CHUNK, dtype=np.float32), np.ones((1, P), np.float32))
    E00 = np.zeros((CHUNK, CHUNK), dtype=np.float32)
    E00[:, 0] = 1.0                      # out row 0 = column sums
    E1 = np.zeros((CHUNK, CHUNK), dtype=np.float32)
    E1[0, 0] = 1.0                       # out row 0 += in row 0
    return U, IND, SU, ONES8, SEL, E00, E1


def _build():
    nc = bacc.Bacc("TRN2", target_bir_lowering=False, debug=False,
                   num_devices=NCORES)
    q_d = nc.declare_dram_parameter("queries", [L, FREE], F32, isOutput=False)
    k_d = nc.declare_dram_parameter("keys", [L, FREE], F32, isOutput=False)
    v_d = nc.declare_dram_parameter("values", [L, FREE], F32, isOutput=False)
    m_d = nc.declare_dram_parameter("mask", [L, 1], F32, isOutput=False)
    u_d = nc.declare_dram_parameter("U", [P, P], F32R, isOutput=False)
    ind_d = nc.declare_dram_parameter("IND", [P, CHUNK * CHUNK], F32R, isOutput=False)
    su_d = nc.declare_dram_parameter("SU", [CHUNK, CHUNK], F32R, isOutput=False)
    on_d = nc.declare_dram_parameter("ONES8", [CHUNK, CHUNK], F32R, isOutput=False)
    sel_d = nc.declare_dram_parameter("SEL", [CHUNK, CHUNK * P], F32R, isOutput=False)
    e00_d = nc.declare_dram_parameter("E00", [CHUNK, CHUNK], F32R, isOutput=False)
    e1_d = nc.declare_dram_parameter("E1", [CHUNK, CHUNK], F32R, isOutput=False)
    zb_d = nc.declare_dram_parameter("ZB", [CHUNK, FREE], F32R, isOutput=False)
    o_d = nc.declare_dram_parameter("out", [L, FREE], F32, isOutput=True)

    with ExitStack() as ctx:
        tc = ctx.enter_context(tile.TileContext(nc))
        const = ctx.enter_context(tc.tile_pool(name="const", bufs=1))
        pk = ctx.enter_context(tc.tile_pool(name="k", bufs=6))
        pkexp = ctx.enter_context(tc.tile_pool(name="kexp", bufs=2 * CHUNK))
        pv = ctx.enter_context(tc.tile_pool(name="v", bufs=6))
        pq = ctx.enter_context(tc.tile_pool(name="q", bufs=6))
        pqs = ctx.enter_context(tc.tile_pool(name="qs", bufs=4))
        pr = ctx.enter_context(tc.tile_pool(name="r", bufs=4))
        pt = ctx.enter_context(tc.tile_pool(name="tkv", bufs=4))
        pt2 = ctx.enter_context(tc.tile_pool(name="t2", bufs=2 * CHUNK))
        po = ctx.enter_context(tc.tile_pool(name="o", bufs=6))
        psmall = ctx.enter_context(tc.tile_pool(name="small", bufs=6))
        pers = ctx.enter_context(tc.tile_pool(name="pers", bufs=1))
        ps_big = ctx.enter_context(tc.tile_pool(name="ps_big", bufs=4, space="PSUM"))
        ps_sm = ctx.enter_context(tc.tile_pool(name="ps_sm", bufs=1, space="PSUM"))

        U = const.tile([P, P], F32R, name="U")
        nc.sync.dma_start(U[:], u_d[:])
        IND = const.tile([P, CHUNK * CHUNK], F32R, name="IND")
        nc.sync.dma_start(IND[:], ind_d[:])
        SU = const.tile([CHUNK, CHUNK], F32R, name="SU")
        nc.sync.dma_start(SU[:], su_d[:])
        ONES8 = const.tile([CHUNK, CHUNK], F32R, name="ONES8")
        nc.sync.dma_start(ONES8[:], on_d[:])
        SEL = const.tile([CHUNK, CHUNK * P], F32R, name="SEL")
        nc.sync.dma_start(SEL[:], sel_d[:])
        E00 = const.tile([CHUNK, CHUNK], F32R, name="E00")
        nc.sync.dma_start(E00[:], e00_d[:])
        E1 = const.tile([CHUNK, CHUNK], F32R, name="E1")
        nc.sync.dma_start(E1[:], e1_d[:])
        MB = const.tile([P, NTILES], F32, name="MB")
        nc.sync.dma_start(MB[:], m_d.rearrange("(t p) o -> p (t o)", p=P))

        # Running scan bases; only row 0 is ever non-zero.
        B1 = pers.tile([CHUNK, FREE], F32R, name="B1")
        nc.sync.dma_start(B1[:], zb_d[:])
        B2 = pers.tile([CHUNK, FREE], F32R, name="B2")
        nc.sync.dma_start(B2[:], zb_d[:])

        for c in range(NCHUNKS):
            # ---- phase A: load K, exp, and accumulate per-tile column sums
            kexp_tiles = []
            scol1 = ps_sm.tile([CHUNK, FREE], F32, tag="scol1")
            for j in range(CHUNK):
                t = c * CHUNK + j
                rows = slice(t * P, (t + 1) * P)
                kt = pk.tile([P, FREE], F32, tag="k")
                nc.sync.dma_start(kt[:], k_d[rows, :])
                ke = pkexp.tile([P, FREE], F32R, tag="kexp")
                nc.scalar.activation(ke[:], kt[:], AF.Exp, bias=MB[:, t:t + 1])
                nc.tensor.matmul(
                    scol1[:],
                    IND[:, j * CHUNK:(j + 1) * CHUNK],
                    ke[:],
                    start=(j == 0),
                    stop=(j == CHUNK - 1),
                )
                kexp_tiles.append(ke)

            # ---- phase B: chunk carry for scan 1
            scol1_sb = psmall.tile([CHUNK, FREE], F32R, tag="scol1_sb")
            nc.scalar.copy(scol1_sb[:], scol1[:])
            c1_ps = ps_sm.tile([CHUNK, FREE], F32, tag="c1")
            nc.tensor.matmul(c1_ps[:], SU[:], scol1_sb[:], start=True, stop=False)
            nc.tensor.matmul(c1_ps[:], ONES8[:], B1[:], start=False, stop=True)
            c1_sb = psmall.tile([CHUNK, FREE], F32R, tag="c1_sb")
            nc.scalar.copy(c1_sb[:], c1_ps[:])
            b1_ps = ps_sm.tile([CHUNK, FREE], F32, tag="c1")
            nc.tensor.matmul(b1_ps[:], E00[:], scol1_sb[:], start=True, stop=False)
            nc.tensor.matmul(b1_ps[:], E1[:], B1[:], start=False, stop=True)
            nc.scalar.copy(B1[:], b1_ps[:])

            # ---- phase C: full scan1 per tile, recip, K*V, T2; scan2 col sums
            t2_tiles = []
            scol2 = ps_sm.tile([CHUNK, FREE], F32, tag="scol2")
            for j in range(CHUNK):
                t = c * CHUNK + j
                rows = slice(t * P, (t + 1) * P)
                s_ps = ps_big.tile([P, FREE], F32, tag="ps_big")
                nc.tensor.matmul(s_ps[:], U[:], kexp_tiles[j][:], start=True, stop=False)
                nc.tensor.matmul(
                    s_ps[:], SEL[:, j * P:(j + 1) * P], c1_sb[:],
                    start=False, stop=True,
                )
                rt = pr.tile([P, FREE], F32, tag="r")
                nc.vector.reciprocal_approx_fast(rt[:], s_ps[:])
                vt = pv.tile([P, FREE], F32, tag="v")
                nc.gpsimd.dma_start(vt[:], v_d[rows, :])
                tkv = pt.tile([P, FREE], F32, tag="tkv")
                nc.gpsimd.tensor_mul(tkv[:], kexp_tiles[j][:].bitcast(F32), vt[:])
                t2 = pt2.tile([P, FREE], F32R, tag="t2")
                nc.vector.tensor_mul(t2[:], tkv[:], rt[:])
                nc.tensor.matmul(
                    scol2[:],
                    IND[:, j * CHUNK:(j + 1) * CHUNK],
                    t2[:],
                    start=(j == 0),
                    stop=(j == CHUNK - 1),
                )
                t2_tiles.append(t2)

            # ---- phase D: chunk carry for scan 2
            scol2_sb = psmall.tile([CHUNK, FREE], F32R, tag="scol2_sb")
            nc.scalar.copy(scol2_sb[:], scol2[:])
            c2_ps = ps_sm.tile([CHUNK, FREE], F32, tag="c2")
            nc.tensor.matmul(c2_ps[:], SU[:], scol2_sb[:], start=True, stop=False)
            nc.tensor.matmul(c2_ps[:], ONES8[:], B2[:], start=False, stop=True)
            c2_sb = psmall.tile([CHUNK, FREE], F32R, tag="c2_sb")
            nc.scalar.copy(c2_sb[:], c2_ps[:])
            b2_ps = ps_sm.tile([CHUNK, FREE], F32, tag="c2")
            nc.tensor.matmul(b2_ps[:], E00[:], scol2_sb[:], start=True, stop=False)
            nc.tensor.matmul(b2_ps[:], E1[:], B2[:], start=False, stop=True)
            nc.scalar.copy(B2[:], b2_ps[:])

            # ---- phase E: full scan2 per tile, sigmoid(Q), output
            for j in range(CHUNK):
                t = c * CHUNK + j
                rows = slice(t * P, (t + 1) * P)
                w_ps = ps_big.tile([P, FREE], F32, tag="ps_big")
                nc.tensor.matmul(w_ps[:], U[:], t2_tiles[j][:], start=True, stop=False)
                nc.tensor.matmul(
                    w_ps[:], SEL[:, j * P:(j + 1) * P], c2_sb[:],
                    start=False, stop=True,
                )
                qt = pq.tile([P, FREE], F32, tag="q")
                nc.gpsimd.dma_start(qt[:], q_d[rows, :])
                qs = pqs.tile([P, FREE], F32, tag="qs")
                nc.scalar.activation(qs[:], qt[:], AF.Sigmoid)
                ot = po.tile([P, FREE], F32, tag="o")
                nc.vector.tensor_mul(ot[:], qs[:], w_ps[:])
                nc.sync.dma_start(o_d[rows, :], ot[:])
    nc.compile()
    return nc


def _get_nc():
    if "nc" not in _CACHE:
        _CACHE["nc"] = _build()
    return _CACHE["nc"]


def _run(queries, keys, values, key_lengths_add, trace=False, **kw):
    nc = _get_nc()
    U, IND, SU, ONES8, SEL, E00, E1 = _constants()
    in_maps = []
    for c in range(NCORES):
        n = c // 2
        h0 = (c % 2) * HPC
        in_maps.append({
            "queries": np.ascontiguousarray(
                queries[n, :, h0:h0 + HPC, :]).reshape(L, FREE),
            "keys": np.ascontiguousarray(
                keys[n, :, h0:h0 + HPC, :]).reshape(L, FREE),
            "values": np.ascontiguousarray(
                values[n, :, h0:h0 + HPC, :]).reshape(L, FREE),
            "mask": np.ascontiguousarray(key_lengths_add[n]).reshape(L, 1),
            "U": U, "IND": IND, "SU": SU, "ONES8": ONES8, "SEL": SEL,
            "E00": E00, "E1": E1,
            "ZB": np.zeros((CHUNK, FREE), dtype=np.float32),
        })
    res = run_bass_kernel_spmd(nc, in_maps, core_ids=list(range(NCORES)),
                               trace=trace, **kw)
    out = np.empty((N, L, H, E), dtype=np.float32)
    for c in range(NCORES):
        n = c // 2
        h0 = (c % 2) * HPC
        out[n, :, h0:h0 + HPC, :] = res.results[c]["out"].reshape(L, HPC, E)
    return out, res


def kernel(queries, keys, values, key_lengths_add):
    out, _ = _run(queries, keys, values, key_lengths_add)
    return out


if __name__ == "__main__":
    rng = np.random.default_rng(0)
    q = rng.standard_normal((N, L, H, E), dtype=np.float32)
    k = rng.standard_normal((N, L, H, E), dtype=np.float32)
    v = rng.standard_normal((N, L, H, E), dtype=np.float32)
    m = np.zeros((N, L), dtype=np.float32)
    o = kernel(q, k, v, m)
    print(o.shape, o.dtype, np.abs(o).mean())


# revision 18
# speedup vs baseline: 1.2697x; 1.2697x over previous
"""AFT simple attention (causal branch) on 8 TRN2 NeuronCores.

out = sigmoid(Q) * cumsum_L( (exp(K+mask) / cumsum_L(exp(K+mask))) * V )

Sharding: data-parallel over (batch n, head-block). Core c handles
n = c // 2 and heads [8*(c%2), 8*(c%2)+8).  Per-core shard = contiguous
[L=8192, 8*64=512] f32 slab per tensor; no cross-core communication.

On-chip layout: L on partitions (128 per tile), (h, e) on the free dim
(512).  All DMAs are fully contiguous.  The cumsum along L is computed
on the TensorEngine as triangular matmuls:
  - per tile: in-block inclusive scan  S_loc = U^T @ X   (U upper-tri ones)
  - per 8-tile chunk: column sums via indicator matmuls, an exclusive
    prefix over the 8 tile-sums via a small strictly-upper matmul, plus
    a running base B; the per-tile carry is broadcast into the same PSUM
    accumulation via selector matmuls.
Max-subtraction is skipped: Kw = exp(K-M)/cumsum(exp(K-M)) is exactly
invariant to M, and |K| <= ~6 so exp stays well within f32 range.
"""

from contextlib import ExitStack

import numpy as np

import concourse.bass as bass
import concourse.tile as tile
from concourse import bacc, mybir
from concourse.bass_utils import run_bass_kernel_spmd

N, L, H, E = 4, 8192, 16, 64
NCORES = 8
HPC = H // 2            # heads per core
FREE = HPC * E          # 512
P = 128                 # L positions per tile
NTILES = L // P         # 64
CHUNK = 8               # tiles per carry chunk
NCHUNKS = NTILES // CHUNK

F32 = mybir.dt.float32
F32R = mybir.dt.float32r
AF = mybir.ActivationFunctionType

_CACHE = {}


def _constants():
    U = np.triu(np.ones((P, P), dtype=np.float32))                 # U[j,l]=1 if j<=l
    SEL127 = np.zeros((P, P), dtype=np.float32)
    SEL127[127, :] = 1.0                 # broadcast partition 127
    return U, SEL127


def _build():
    nc = bacc.Bacc("TRN2", target_bir_lowering=False, debug=False,
                   num_devices=NCORES)
    q_d = nc.declare_dram_parameter("queries", [L, FREE], F32, isOutput=False)
    k_d = nc.declare_dram_parameter("keys", [L, FREE], F32R, isOutput=False)
    v_d = nc.declare_dram_parameter("values", [L, FREE], F32R, isOutput=False)
    m_d = nc.declare_dram_parameter("mask", [L, 1], F32, isOutput=False)
    u_d = nc.declare_dram_parameter("U", [P, P], F32R, isOutput=False)
    ind_d = nc.declare_dram_parameter("IND", [P, CHUNK * CHUNK], F32R, isOutput=False)
    su_d = nc.declare_dram_parameter("SU", [CHUNK, CHUNK], F32R, isOutput=False)
    on_d = nc.declare_dram_parameter("ONES8", [CHUNK, CHUNK], F32R, isOutput=False)
    sel_d = nc.declare_dram_parameter("SEL", [CHUNK, CHUNK * P], F32R, isOutput=False)
    e00_d = nc.declare_dram_parameter("E00", [CHUNK, CHUNK], F32R, isOutput=False)
    e1_d = nc.declare_dram_parameter("E1", [CHUNK, CHUNK], F32R, isOutput=False)
    zb_d = nc.declare_dram_parameter("ZB", [CHUNK, FREE], F32R, isOutput=False)
    o_d = nc.declare_dram_parameter("out", [L, FREE], F32, isOutput=True)

    with ExitStack() as ctx:
        tc = ctx.enter_context(tile.TileContext(nc))
        const = ctx.enter_context(tc.tile_pool(name="const", bufs=1))
        pk = ctx.enter_context(tc.tile_pool(name="k", bufs=5))
        pkexp = ctx.enter_context(tc.tile_pool(name="kexp", bufs=2 * CHUNK))
        pv = ctx.enter_context(tc.tile_pool(name="v", bufs=5))
        pt = ctx.enter_context(tc.tile_pool(name="tkv", bufs=3))
        pq = ctx.enter_context(tc.tile_pool(name="q", bufs=6))
        pqs = ctx.enter_context(tc.tile_pool(name="qs", bufs=4))
        pr = ctx.enter_context(tc.tile_pool(name="r", bufs=4))
        pt = ctx.enter_context(tc.tile_pool(name="tkv", bufs=4))
        pt2 = ctx.enter_context(tc.tile_pool(name="t2", bufs=2 * CHUNK))
        po = ctx.enter_context(tc.tile_pool(name="o", bufs=6))
        psmall = ctx.enter_context(tc.tile_pool(name="small", bufs=2))
        pstg = ctx.enter_context(tc.tile_pool(name="stg", bufs=3))
        pers = ctx.enter_context(tc.tile_pool(name="pers", bufs=1))
        ps_big = ctx.enter_context(tc.tile_pool(name="ps_big", bufs=6, space="PSUM"))
        ps_sm = ctx.enter_context(tc.tile_pool(name="ps_sm", bufs=1, space="PSUM"))

        U = const.tile([P, P], F32R, name="U")
        nc.sync.dma_start(U[:], u_d[:])
        IND = const.tile([P, CHUNK * CHUNK], F32R, name="IND")
        nc.sync.dma_start(IND[:], ind_d[:])
        SU = const.tile([CHUNK, CHUNK], F32R, name="SU")
        nc.sync.dma_start(SU[:], su_d[:])
        ONES8 = const.tile([CHUNK, CHUNK], F32R, name="ONES8")
        nc.sync.dma_start(ONES8[:], on_d[:])
        SEL = const.tile([CHUNK, CHUNK * P], F32R, name="SEL")
        nc.sync.dma_start(SEL[:], sel_d[:])
        E00 = const.tile([CHUNK, CHUNK], F32R, name="E00")
        nc.sync.dma_start(E00[:], e00_d[:])
        E1 = const.tile([CHUNK, CHUNK], F32R, name="E1")
        nc.sync.dma_start(E1[:], e1_d[:])
        MB = const.tile([P, NTILES], F32, name="MB")
        nc.sync.dma_start(MB[:], m_d.rearrange("(t p) o -> p (t o)", p=P))

        # Running scan bases; only row 0 is ever non-zero.
        B1 = pers.tile([CHUNK, FREE], F32R, name="B1")
        nc.sync.dma_start(B1[:], zb_d[:])
        B2 = pers.tile([CHUNK, FREE], F32R, name="B2")
        nc.sync.dma_start(B2[:], zb_d[:])

        for c in range(NCHUNKS):
            # ---- phase A: load K, exp, and accumulate per-tile column sums
            kexp_tiles = []
            scol1 = ps_sm.tile([CHUNK, FREE], F32, tag="scol1")
            for j in range(CHUNK):
                t = c * CHUNK + j
                rows = slice(t * P, (t + 1) * P)
                kt = pk.tile([P, FREE], F32, tag="k")
                nc.sync.dma_start(kt[:], k_d[rows, :])
                ke = pkexp.tile([P, FREE], F32R, tag="kexp")
                nc.scalar.activation(ke[:], kt[:], AF.Exp, bias=MB[:, t:t + 1])
                nc.tensor.matmul(
                    scol1[:],
                    IND[:, j * CHUNK:(j + 1) * CHUNK],
                    ke[:],
                    start=(j == 0),
                    stop=(j == CHUNK - 1),
                )
                kexp_tiles.append(ke)

            # ---- phase B: chunk carry for scan 1
            scol1_sb = psmall.tile([CHUNK, FREE], F32R, tag="scol1_sb")
            nc.scalar.copy(scol1_sb[:], scol1[:])
            c1_ps = ps_sm.tile([CHUNK, FREE], F32, tag="scol1", name="c1_ps")
            nc.tensor.matmul(c1_ps[:], SU[:], scol1_sb[:], start=True, stop=False)
            nc.tensor.matmul(c1_ps[:], ONES8[:], B1[:], start=False, stop=True)
            c1_sb = psmall.tile([CHUNK, FREE], F32R, tag="c1_sb")
            nc.scalar.copy(c1_sb[:], c1_ps[:])
            b1_ps = ps_sm.tile([CHUNK, FREE], F32, tag="c1")
            nc.tensor.matmul(b1_ps[:], E00[:], scol1_sb[:], start=True, stop=False)
            nc.tensor.matmul(b1_ps[:], E1[:], B1[:], start=False, stop=True)
            nc.scalar.copy(B1[:], b1_ps[:])

            # ---- phase C: full scan1 per tile, recip, K*V, T2; scan2 col sums
            t2_tiles = []
            scol2 = ps_sm.tile([CHUNK, FREE], F32, tag="scol2")
            for j in range(CHUNK):
                t = c * CHUNK + j
                rows = slice(t * P, (t + 1) * P)
                s_ps = ps_big.tile([P, FREE], F32, tag="ps_big")
                nc.tensor.matmul(s_ps[:], U[:], kexp_tiles[j][:], start=True, stop=False)
                nc.tensor.matmul(
                    s_ps[:], SEL[:, j * P:(j + 1) * P], c1_sb[:],
                    start=False, stop=True,
                )
                rt = pr.tile([P, FREE], F32, tag="r")
                nc.vector.reciprocal_approx_fast(rt[:], s_ps[:])
                vt = pv.tile([P, FREE], F32, tag="v")
                nc.gpsimd.dma_start(vt[:], v_d[rows, :])
                tkv = pt.tile([P, FREE], F32, tag="tkv")
                nc.gpsimd.tensor_mul(tkv[:], kexp_tiles[j][:].bitcast(F32), vt[:])
                t2 = pt2.tile([P, FREE], F32R, tag="t2")
                nc.vector.tensor_mul(t2[:], tkv[:], rt[:])
                nc.tensor.matmul(
                    scol2[:],
                    IND[:, j * CHUNK:(j + 1) * CHUNK],
                    t2[:],
                    start=(j == 0),
                    stop=(j == CHUNK - 1),
                )
                t2_tiles.append(t2)

            # ---- phase D: chunk carry for scan 2
            scol2_sb = psmall.tile([CHUNK, FREE], F32R, tag="scol2_sb")
            nc.scalar.copy(scol2_sb[:], scol2[:])
            c2_ps = ps_sm.tile([CHUNK, FREE], F32, tag="scol2", name="c2_ps")
            nc.tensor.matmul(c2_ps[:], SU[:], scol2_sb[:], start=True, stop=False)
            nc.tensor.matmul(c2_ps[:], ONES8[:], B2[:], start=False, stop=True)
            c2_sb = psmall.tile([CHUNK, FREE], F32R, tag="c2_sb")
            nc.scalar.copy(c2_sb[:], c2_ps[:])
            b2_ps = ps_sm.tile([CHUNK, FREE], F32, tag="c2")
            nc.tensor.matmul(b2_ps[:], E00[:], scol2_sb[:], start=True, stop=False)
            nc.tensor.matmul(b2_ps[:], E1[:], B2[:], start=False, stop=True)
            nc.scalar.copy(B2[:], b2_ps[:])

            # ---- phase E: full scan2 per tile, sigmoid(Q), output
            for j in range(CHUNK):
                t = c * CHUNK + j
                rows = slice(t * P, (t + 1) * P)
                w_ps = ps_big.tile([P, FREE], F32, tag="ps_big")
                nc.tensor.matmul(w_ps[:], U[:], t2_tiles[j][:], start=True, stop=False)
                nc.tensor.matmul(
                    w_ps[:], SEL[:, j * P:(j + 1) * P], c2_sb[:],
                    start=False, stop=True,
                )
                qt = pq.tile([P, FREE], F32, tag="q")
                nc.gpsimd.dma_start(qt[:], q_d[rows, :])
                qs = pqs.tile([P, FREE], F32, tag="qs")
                nc.scalar.activation(qs[:], qt[:], AF.Sigmoid)
                ot = po.tile([P, FREE], F32, tag="o")
                nc.vector.tensor_mul(ot[:], qs[:], w_ps[:])
                nc.sync.dma_start(o_d[rows, :], ot[:])
        if pend_e is not None:
            pend_e()
    nc.compile()
    return nc


def _get_nc():
    if "nc" not in _CACHE:
        _CACHE["nc"] = _build()
    return _CACHE["nc"]


def _run(queries, keys, values, key_lengths_add, trace=False, **kw):
    nc = _get_nc()
    U, IND, SU, ONES8, SEL, E00, E1 = _constants()
    in_maps = []
    for c in range(NCORES):
        n = c // 2
        h0 = (c % 2) * HPC
        in_maps.append({
            "queries": np.ascontiguousarray(
                queries[n, :, h0:h0 + HPC, :]).reshape(L, FREE),
            "keys": np.ascontiguousarray(
                keys[n, :, h0:h0 + HPC, :]).reshape(L, FREE),
            "values": np.ascontiguousarray(
                values[n, :, h0:h0 + HPC, :]).reshape(L, FREE),
            "mask": np.ascontiguousarray(key_lengths_add[n]).reshape(L, 1),
            "U": U, "IND": IND, "SU": SU, "ONES8": ONES8, "SEL": SEL,
            "E00": E00, "E1": E1,
            "ZB": np.zeros((CHUNK, FREE), dtype=np.float32),
        })
    res = run_bass_kernel_spmd(nc, in_maps, core_ids=list(range(NCORES)),
                               trace=trace, **kw)
    out = np.empty((N, L, H, E), dtype=np.float32)
    for c in range(NCORES):
        n = c // 2
        h0 = (c % 2) * HPC
        out[n, :, h0:h0 + HPC, :] = res.results[c]["out"].reshape(L, HPC, E)
    return out, res


def kernel(queries, keys, values, key_lengths_add):
    out, _ = _run(queries, keys, values, key_lengths_add)
    return out


if __name__ == "__main__":
    rng = np.random.default_rng(0)
    q = rng.standard_normal((N, L, H, E), dtype=np.float32)
    k = rng.standard_normal((N, L, H, E), dtype=np.float32)
    v = rng.standard_normal((N, L, H, E), dtype=np.float32)
    m = np.zeros((N, L), dtype=np.float32)
    o = kernel(q, k, v, m)
    print(o.shape, o.dtype, np.abs(o).mean())


# revision 19
# speedup vs baseline: 1.3380x; 1.0537x over previous
"""AFT simple attention (causal branch) on 8 TRN2 NeuronCores.

out = sigmoid(Q) * cumsum_L( (exp(K+mask) / cumsum_L(exp(K+mask))) * V )

Sharding: data-parallel over (batch n, head-block). Core c handles
n = c // 2 and heads [8*(c%2), 8*(c%2)+8).  Per-core shard = contiguous
[L=8192, 8*64=512] f32 slab per tensor; no cross-core communication.

On-chip layout: L on partitions (128 per tile), (h, e) on the free dim
(512).  All DMAs are fully contiguous.  The cumsum along L is computed
on the TensorEngine as triangular matmuls:
  - per tile: in-block inclusive scan  S_loc = U^T @ X   (U upper-tri ones)
  - per 8-tile chunk: column sums via indicator matmuls, an exclusive
    prefix over the 8 tile-sums via a small strictly-upper matmul, plus
    a running base B; the per-tile carry is broadcast into the same PSUM
    accumulation via selector matmuls.
Max-subtraction is skipped: Kw = exp(K-M)/cumsum(exp(K-M)) is exactly
invariant to M, and |K| <= ~6 so exp stays well within f32 range.
"""

from contextlib import ExitStack

import numpy as np

import concourse.bass as bass
import concourse.tile as tile
from concourse import bacc, mybir
from concourse.bass_utils import run_bass_kernel_spmd

N, L, H, E = 4, 8192, 16, 64
NCORES = 8
HPC = H // 2            # heads per core
FREE = HPC * E          # 512
P = 128                 # L positions per tile
NTILES = L // P         # 64
CHUNK = 8               # tiles per carry chunk
NCHUNKS = NTILES // CHUNK

F32 = mybir.dt.float32
F32R = mybir.dt.float32r
AF = mybir.ActivationFunctionType

_CACHE = {}


def _constants():
    U = np.triu(np.ones((P, P), dtype=np.float32))                 # U[j,l]=1 if j<=l
    SEL127 = np.zeros((P, P), dtype=np.float32)
    SEL127[127, :] = 1.0                 # broadcast partition 127
    return U, SEL127


def _build():
    nc = bacc.Bacc("TRN2", target_bir_lowering=False, debug=False,
                   num_devices=NCORES)
    q_d = nc.declare_dram_parameter("queries", [L, FREE], F32, isOutput=False)
    k_d = nc.declare_dram_parameter("keys", [L, FREE], F32R, isOutput=False)
    v_d = nc.declare_dram_parameter("values", [L, FREE], F32R, isOutput=False)
    m_d = nc.declare_dram_parameter("mask", [L, 1], F32, isOutput=False)
    u_d = nc.declare_dram_parameter("U", [P, P], F32R, isOutput=False)
    ind_d = nc.declare_dram_parameter("IND", [P, CHUNK * CHUNK], F32R, isOutput=False)
    su_d = nc.declare_dram_parameter("SU", [CHUNK, CHUNK], F32R, isOutput=False)
    on_d = nc.declare_dram_parameter("ONES8", [CHUNK, CHUNK], F32R, isOutput=False)
    sel_d = nc.declare_dram_parameter("SEL", [CHUNK, CHUNK * P], F32R, isOutput=False)
    e00_d = nc.declare_dram_parameter("E00", [CHUNK, CHUNK], F32R, isOutput=False)
    e1_d = nc.declare_dram_parameter("E1", [CHUNK, CHUNK], F32R, isOutput=False)
    zb_d = nc.declare_dram_parameter("ZB", [CHUNK, FREE], F32R, isOutput=False)
    o_d = nc.declare_dram_parameter("out", [L, FREE], F32, isOutput=True)

    with ExitStack() as ctx:
        tc = ctx.enter_context(tile.TileContext(nc))
        const = ctx.enter_context(tc.tile_pool(name="const", bufs=1))
        pk = ctx.enter_context(tc.tile_pool(name="k", bufs=5))
        pkexp = ctx.enter_context(tc.tile_pool(name="kexp", bufs=2 * CHUNK))
        pv = ctx.enter_context(tc.tile_pool(name="v", bufs=5))
        pt = ctx.enter_context(tc.tile_pool(name="tkv", bufs=3))
        pq = ctx.enter_context(tc.tile_pool(name="q", bufs=6))
        pqs = ctx.enter_context(tc.tile_pool(name="qs", bufs=4))
        pr = ctx.enter_context(tc.tile_pool(name="r", bufs=4))
        pt = ctx.enter_context(tc.tile_pool(name="tkv", bufs=4))
        pt2 = ctx.enter_context(tc.tile_pool(name="t2", bufs=2 * CHUNK))
        po = ctx.enter_context(tc.tile_pool(name="o", bufs=6))
        psmall = ctx.enter_context(tc.tile_pool(name="small", bufs=2))
        pers = ctx.enter_context(tc.tile_pool(name="pers", bufs=1))
        ps_big = ctx.enter_context(tc.tile_pool(name="ps_big", bufs=6, space="PSUM"))
        ps_sm = ctx.enter_context(tc.tile_pool(name="ps_sm", bufs=1, space="PSUM"))

        U = const.tile([P, P], F32R, name="U")
        nc.sync.dma_start(U[:], u_d[:])
        IND = const.tile([P, CHUNK * CHUNK], F32R, name="IND")
        nc.sync.dma_start(IND[:], ind_d[:])
        SU = const.tile([CHUNK, CHUNK], F32R, name="SU")
        nc.sync.dma_start(SU[:], su_d[:])
        ONES8 = const.tile([CHUNK, CHUNK], F32R, name="ONES8")
        nc.sync.dma_start(ONES8[:], on_d[:])
        SEL = const.tile([CHUNK, CHUNK * P], F32R, name="SEL")
        nc.sync.dma_start(SEL[:], sel_d[:])
        E00 = const.tile([CHUNK, CHUNK], F32R, name="E00")
        nc.sync.dma_start(E00[:], e00_d[:])
        E1 = const.tile([CHUNK, CHUNK], F32R, name="E1")
        nc.sync.dma_start(E1[:], e1_d[:])
        MB = const.tile([P, NTILES], F32, name="MB")
        nc.sync.dma_start(MB[:], m_d.rearrange("(t p) o -> p (t o)", p=P))

        # Running scan bases; only row 0 is ever non-zero.
        B1 = pers.tile([CHUNK, FREE], F32R, name="B1")
        nc.sync.dma_start(B1[:], zb_d[:])
        B2 = pers.tile([CHUNK, FREE], F32R, name="B2")
        nc.sync.dma_start(B2[:], zb_d[:])

        for c in range(NCHUNKS):
            # ---- phase A: load K, exp, and accumulate per-tile column sums
            kexp_tiles = []
            scol1 = ps_sm.tile([CHUNK, FREE], F32, tag="scol1")
            for j in range(CHUNK):
                t = c * CHUNK + j
                rows = slice(t * P, (t + 1) * P)
                kt = pk.tile([P, FREE], F32, tag="k")
                nc.sync.dma_start(kt[:], k_d[rows, :])
                ke = pkexp.tile([P, FREE], F32R, tag="kexp")
                nc.scalar.activation(ke[:], kt[:], AF.Exp, bias=MB[:, t:t + 1])
                nc.tensor.matmul(
                    scol1[:],
                    IND[:, j * CHUNK:(j + 1) * CHUNK],
                    ke[:],
                    start=(j == 0),
                    stop=(j == CHUNK - 1),
                )
                kexp_tiles.append(ke)

            # ---- phase B: chunk carry for scan 1
            scol1_sb = psmall.tile([CHUNK, FREE], F32R, tag="scol1_sb")
            nc.scalar.copy(scol1_sb[:], scol1[:])
            c1_ps = ps_sm.tile([CHUNK, FREE], F32, tag="scol1", name="c1_ps")
            nc.tensor.matmul(c1_ps[:], SU[:], scol1_sb[:], start=True, stop=False)
            nc.tensor.matmul(c1_ps[:], ONES8[:], B1[:], start=False, stop=True)
            c1_sb = psmall.tile([CHUNK, FREE], F32R, tag="c1_sb")
            nc.scalar.copy(c1_sb[:], c1_ps[:])
            b1_ps = ps_sm.tile([CHUNK, FREE], F32, tag="c1")
            nc.tensor.matmul(b1_ps[:], E00[:], scol1_sb[:], start=True, stop=False)
            nc.tensor.matmul(b1_ps[:], E1[:], B1[:], start=False, stop=True)
            nc.scalar.copy(B1[:], b1_ps[:])

            # ---- phase C: full scan1 per tile, recip, K*V, T2; scan2 col sums
            t2_tiles = []
            scol2 = ps_sm.tile([CHUNK, FREE], F32, tag="scol2")
            for j in range(CHUNK):
                t = c * CHUNK + j
                rows = slice(t * P, (t + 1) * P)
                s_ps = ps_big.tile([P, FREE], F32, tag="ps_big")
                nc.tensor.matmul(s_ps[:], U[:], kexp_tiles[j][:], start=True, stop=False)
                nc.tensor.matmul(
                    s_ps[:], SEL[:, j * P:(j + 1) * P], c1_sb[:],
                    start=False, stop=True,
                )
                rt = pr.tile([P, FREE], F32, tag="r")
                nc.vector.reciprocal_approx_fast(rt[:], s_ps[:])
                vt = pv.tile([P, FREE], F32, tag="v")
                nc.gpsimd.dma_start(vt[:], v_d[rows, :])
                tkv = pt.tile([P, FREE], F32, tag="tkv")
                nc.gpsimd.tensor_mul(tkv[:], kexp_tiles[j][:].bitcast(F32), vt[:])
                t2 = pt2.tile([P, FREE], F32R, tag="t2")
                nc.vector.tensor_mul(t2[:], tkv[:], rt[:])
                nc.tensor.matmul(
                    scol2[:],
                    IND[:, j * CHUNK:(j + 1) * CHUNK],
                    t2[:],
                    start=(j == 0),
                    stop=(j == CHUNK - 1),
                )
                t2_tiles.append(t2)

            # ---- phase D: chunk carry for scan 2
            scol2_sb = psmall.tile([CHUNK, FREE], F32R, tag="scol2_sb")
            nc.scalar.copy(scol2_sb[:], scol2[:])
            c2_ps = ps_sm.tile([CHUNK, FREE], F32, tag="scol2", name="c2_ps")
            nc.tensor.matmul(c2_ps[:], SU[:], scol2_sb[:], start=True, stop=False)
            nc.tensor.matmul(c2_ps[:], ONES8[:], B2[:], start=False, stop=True)
            c2_sb = psmall.tile([CHUNK, FREE], F32R, tag="c2_sb")
            nc.scalar.copy(c2_sb[:], c2_ps[:])
            b2_ps = ps_sm.tile([CHUNK, FREE], F32, tag="c2")
            nc.tensor.matmul(b2_ps[:], E00[:], scol2_sb[:], start=True, stop=False)
            nc.tensor.matmul(b2_ps[:], E1[:], B2[:], start=False, stop=True)
            nc.scalar.copy(B2[:], b2_ps[:])

            # ---- phase E: full scan2 per tile, sigmoid(Q), output
            for j in range(CHUNK):
                t = c * CHUNK + j
                rows = slice(t * P, (t + 1) * P)
                w_ps = ps_big.tile([P, FREE], F32, tag="ps_big")
                nc.tensor.matmul(w_ps[:], U[:], t2_tiles[j][:], start=True, stop=False)
                nc.tensor.matmul(
                    w_ps[:], SEL[:, j * P:(j + 1) * P], c2_sb[:],
                    start=False, stop=True,
                )
                qt = pq.tile([P, FREE], F32, tag="q")
                nc.gpsimd.dma_start(qt[:], q_d[rows, :])
                qs = pqs.tile([P, FREE], F32, tag="qs")
                nc.scalar.activation(qs[:], qt[:], AF.Sigmoid)
                ot = po.tile([P, FREE], F32, tag="o")
                nc.vector.tensor_mul(ot[:], qs[:], w_ps[:])
                nc.sync.dma_start(o_d[rows, :], ot[:])
        if pend_e is not None:
            pend_e()
    nc.compile()
    return nc


def _get_nc():
    if "nc" not in _CACHE:
        _CACHE["nc"] = _build()
    return _CACHE["nc"]


def _run(queries, keys, values, key_lengths_add, trace=False, **kw):
    nc = _get_nc()
    U, IND, SU, ONES8, SEL, E00, E1 = _constants()
    in_maps = []
    for c in range(NCORES):
        n = c // 2
        h0 = (c % 2) * HPC
        in_maps.append({
            "queries": np.ascontiguousarray(
                queries[n, :, h0:h0 + HPC, :]).reshape(L, FREE),
            "keys": np.ascontiguousarray(
                keys[n, :, h0:h0 + HPC, :]).reshape(L, FREE),
            "values": np.ascontiguousarray(
                values[n, :, h0:h0 + HPC, :]).reshape(L, FREE),
            "mask": np.ascontiguousarray(key_lengths_add[n]).reshape(L, 1),
            "U": U, "IND": IND, "SU": SU, "ONES8": ONES8, "SEL": SEL,
            "E00": E00, "E1": E1,
            "ZB": np.zeros((CHUNK, FREE), dtype=np.float32),
        })
    res = run_bass_kernel_spmd(nc, in_maps, core_ids=list(range(NCORES)),
                               trace=trace, **kw)
    out = np.empty((N, L, H, E), dtype=np.float32)
    for c in range(NCORES):
        n = c // 2
        h0 = (c % 2) * HPC
        out[n, :, h0:h0 + HPC, :] = res.results[c]["out"].reshape(L, HPC, E)
    return out, res


def kernel(queries, keys, values, key_lengths_add):
    out, _ = _run(queries, keys, values, key_lengths_add)
    return out


if __name__ == "__main__":
    rng = np.random.default_rng(0)
    q = rng.standard_normal((N, L, H, E), dtype=np.float32)
    k = rng.standard_normal((N, L, H, E), dtype=np.float32)
    v = rng.standard_normal((N, L, H, E), dtype=np.float32)
    m = np.zeros((N, L), dtype=np.float32)
    o = kernel(q, k, v, m)
    print(o.shape, o.dtype, np.abs(o).mean())
